# revision 11
# baseline (speedup 1.0000x reference)
"""Trainium2 Bass kernel for a 12-layer autoregressive transformer.

Sharding: 4 batch elements x 2-way sequence split across 8 cores.
Core pair p = (2p, 2p+1) handles batch element p. Within a pair, core
half 0 owns 128-token blocks [0,3,4,7], half 1 owns [1,2,5,6] (this
balances causal-attention work exactly: 18 block-pairs each).

Per layer, the two cores of a pair exchange the LN1 output xhat
(bf16, 512KB) with one AllGather launched right after LN1; each core
then computes the peer's K/V projections locally from the gathered
xhat (cheaper than exchanging K+V, and the collective overlaps the
local QKV projections + the local half of attention). Attention runs
in two passes: local key blocks for all heads first (no dependency on
the collective), then remote key blocks.

On-device layout is feature-major (features on SBUF partitions, tokens
on the free axis). All weights are bf16 (same PE rate as f32r, half
the HBM traffic), prefetched one layer ahead. The residual stream h
stays fp32. LN statistics are computed via PE column-sum matmuls; the
per-token 1/sqrt(var) uses a DVE bit-trick + 2 Newton steps so the
scalar engine's activation table never swaps for LN (only exp<->gelu
swaps remain). Softmax denominators come from a ones-column appended
to V; per-token (free-axis) broadcasts are K=1 matmuls on the PE.
"""

import os
import numpy as np
import ml_dtypes

import concourse.bass as bass
import concourse.mybir as mybir
import concourse.tile as tile
from concourse import bacc
from concourse.bass_utils import run_bass_kernel_spmd

F32 = mybir.dt.float32
F32R = mybir.dt.float32r
BF16 = mybir.dt.bfloat16
I32 = mybir.dt.int32

S, D, H, HD, L, DFF, VOCAB = 1024, 512, 8, 64, 12, 2048, 19
SCHEMA, NDIMS = 21, 64
NB, TB = 8, 128            # token blocks of 128
TLOC = 512                 # tokens per core
DC = D // 128              # 4 feature chunks
H0_BLOCKS = [0, 3, 4, 7]
H1_BLOCKS = [1, 2, 5, 6]
# padded q-window widths per key block (max over the two halves' suffix counts)
# virtual attention slots: 4 local blocks then 4 remote (peer) blocks, each
# ordered ascending; q-window width for slot s is (4 - s) * 128 padded to the
# max over halves -- identical for both halves by construction of the split.
NPAD_V = [512, 384, 256, 128, 512, 384, 256, 128]
OFF = np.concatenate([[0], np.cumsum(NPAD_V)]).astype(int)
SUM_NPAD = int(OFF[-1])                # 2816

X_BF16 = DC * 128 * TLOC               # xhat half, bf16 elems (262144)
X_WORDS = X_BF16 // 2                  # f32r words in bounce
PAIRS = [[0, 1], [2, 3], [4, 5], [6, 7]]
RSQRT_MAGIC = 0x5F3759DF

_PROGRAM_CACHE = {}
LAST_RESULTS = None
LAST_EXEC_S = None


def _run_spmd(nc, in_maps, n_cores=8, bench_reps=0):
    """Execute a prebuilt Bass module on 8 cores via PJRT (axon), jitting
    once; optionally re-run the warm executable to measure execution time."""
    global LAST_EXEC_S
    import time
    import jax
    from jax.experimental.shard_map import shard_map
    from jax.sharding import Mesh, PartitionSpec
    from concourse import bass2jax, mybir as _mybir
    bass2jax.install_neuronx_cc_hook()

    partition_name = nc.partition_id_tensor.name if nc.partition_id_tensor else None
    in_names, out_names, out_avals, zero_outs = [], [], [], []
    for alloc in nc.m.functions[0].allocations:
        if not isinstance(alloc, _mybir.MemoryLocationSet):
            continue
        name = alloc.memorylocations[0].name
        if alloc.kind == "ExternalInput":
            if name != partition_name:
                in_names.append(name)
        elif alloc.kind == "ExternalOutput":
            shape = tuple(alloc.tensor_shape)
            dtype = _mybir.dt.np(alloc.dtype)
            out_names.append(name)
            out_avals.append(jax.core.ShapedArray(shape, dtype))
            zero_outs.append(np.zeros(shape, dtype))
    n_params = len(in_names)
    n_outs = len(out_avals)
    all_in_names = list(in_names) + list(out_names)
    if partition_name is not None:
        all_in_names.append(partition_name)

    def _body(*args):
        operands = list(args)
        if partition_name is not None:
            operands.append(bass2jax.partition_id_tensor())
        outs = bass2jax._bass_exec_p.bind(
            *operands, out_avals=tuple(out_avals), in_names=tuple(all_in_names),
            out_names=tuple(out_names), lowering_input_output_aliases=(),
            sim_require_finite=True, sim_require_nnan=True, nc=nc)
        return tuple(outs)

    devices = jax.devices()[:n_cores]
    mesh = Mesh(np.asarray(devices), ("core",))
    in_specs = (PartitionSpec("core"),) * (n_params + n_outs)
    out_specs = (PartitionSpec("core"),) * n_outs
    donate = tuple(range(n_params, n_params + n_outs))
    sharded = jax.jit(
        shard_map(_body, mesh=mesh, in_specs=in_specs, out_specs=out_specs,
                  check_rep=False),
        donate_argnums=donate, keep_unused=True)

    concat_in = [np.concatenate([np.asarray(in_maps[c][nm])[None]
                                 for c in range(n_cores)], axis=0)
                 .reshape(n_cores * np.asarray(in_maps[0][nm]).shape[0],
                          *np.asarray(in_maps[0][nm]).shape[1:])
                 for nm in in_names]
    def _zeros():
        return [np.zeros((n_cores * z.shape[0], *z.shape[1:]), z.dtype)
                for z in zero_outs]

    out_arrs = jax.block_until_ready(sharded(*concat_in, *_zeros()))

    if bench_reps:
        from jax.sharding import NamedSharding
        shardings = [NamedSharding(mesh, PartitionSpec("core"))] * len(concat_in)
        dev_in = [jax.device_put(a, s) for a, s in zip(concat_in, shardings)]
        jax.block_until_ready(dev_in)
        times = []
        for _ in range(bench_reps):
            zo = [jax.device_put(z, NamedSharding(mesh, PartitionSpec("core")))
                  for z in _zeros()]
            jax.block_until_ready(zo)
            t0 = time.perf_counter()
            r = jax.block_until_ready(sharded(*dev_in, *zo))
            times.append(time.perf_counter() - t0)
        LAST_EXEC_S = min(times)

    return [{nm: np.asarray(out_arrs[i]).reshape(n_cores, *out_avals[i].shape)[c]
             for i, nm in enumerate(out_names)} for c in range(n_cores)]


def _build_program(n_layers=L, fake_ag=False):
    nc = bacc.Bacc("TRN2", target_bir_lowering=False, num_devices=8)

    # ---------------- DRAM I/O ----------------
    xsT_d = nc.dram_tensor("xsT", [NDIMS, TLOC], F32R, kind="ExternalInput")
    posT_d = nc.dram_tensor("posT", [DC, 128, TLOC], F32, kind="ExternalInput")
    masks_d = nc.dram_tensor("masks", [2, 128, SUM_NPAD], BF16, kind="ExternalInput")
    rw_d = nc.dram_tensor("rw", [NDIMS, D], F32R, kind="ExternalInput")
    w1_d = nc.dram_tensor("w1", [L, D, 3 * D], BF16, kind="ExternalInput")
    wp_d = nc.dram_tensor("wp", [L, D, D], BF16, kind="ExternalInput")
    w2_d = nc.dram_tensor("w2", [L, D, DFF], BF16, kind="ExternalInput")
    w3_d = nc.dram_tensor("w3", [L, DFF, D], BF16, kind="ExternalInput")
    wo_d = nc.dram_tensor("wo", [D, VOCAB], BF16, kind="ExternalInput")
    out_d = nc.dram_tensor("outT", [VOCAB, TLOC], F32, kind="ExternalOutput")

    bounce = nc.dram_tensor("bounce", [X_WORDS], F32R)
    agout = nc.dram_tensor("agout", [2 * X_WORDS], F32R)

    with tile.TileContext(nc) as tc:
        _emit(nc, tc, locals(), n_layers, fake_ag)
    nc.compile()
    return nc


def _emit(nc, tc, d, n_layers, fake_ag=False):
    xsT_d, posT_d, masks_d, rw_d = d["xsT_d"], d["posT_d"], d["masks_d"], d["rw_d"]
    w1_d, wp_d, w2_d, w3_d, wo_d = d["w1_d"], d["wp_d"], d["w2_d"], d["w3_d"], d["wo_d"]
    out_d, bounce, agout = d["out_d"], d["bounce"], d["agout"]
    AF = mybir.ActivationFunctionType
    Alu = mybir.AluOpType

    import contextlib
    ctx = contextlib.ExitStack()
    persist = ctx.enter_context(tc.tile_pool(name="persist", bufs=1))
    scr = ctx.enter_context(tc.tile_pool(name="scr", bufs=1))
    wpool = ctx.enter_context(tc.tile_pool(name="wpool", bufs=14))
    ppool = ctx.enter_context(tc.tile_pool(name="ppool", bufs=10))
    small = ctx.enter_context(tc.tile_pool(name="small", bufs=4))
    opool = ctx.enter_context(tc.tile_pool(name="opool", bufs=1))
    ps_mm = ctx.enter_context(tc.tile_pool(name="ps_mm", bufs=4, space="PSUM"))
    ps_cps = ctx.enter_context(tc.tile_pool(name="ps_cps", bufs=2, space="PSUM"))
    ps_misc = ctx.enter_context(tc.tile_pool(name="ps_misc", bufs=2, space="PSUM"))

    # ---- persistent tiles ----
    h = persist.tile([128, DC, TLOC], F32R)
    xhat = persist.tile([128, DC, TLOC], BF16)
    xrem = persist.tile([128, DC, TLOC], BF16)
    qT = persist.tile([128, DC, TLOC], BF16)
    kst = persist.tile([128, DC, TLOC], BF16)
    kpe = persist.tile([128, DC, TLOC], BF16)
    vloc = persist.tile([128, 4, H, HD + 1], BF16)
    vpe = persist.tile([128, 4, H, HD + 1], BF16)
    ctxl = persist.tile([HD + 1, H, TLOC], F32)
    ctxf = persist.tile([128, DC, TLOC], BF16)
    mt = persist.tile([128, 2, SUM_NPAD], BF16)
    gel = persist.tile([128, 16, TLOC], BF16)
    x2s = persist.tile([128, DC, TLOC], F32R)   # squares for LN stats
    onesc = persist.tile([128, 1], F32R)        # 1/512 column (mean via matmul)
    onesr = persist.tile([1, TLOC], F32R)       # exact ones row
    epst = persist.tile([1, 1], F32R)           # eps value for var
    scr1 = persist.tile([1, 1], F32)            # scratch for act-table preloads
    xsT = persist.tile([NDIMS, TLOC], F32R)
    rw = persist.tile([NDIMS, D], F32R)

    nc.vector.memset(onesc[:].bitcast(F32), 1.0 / D)
    nc.vector.memset(onesr[:].bitcast(F32), 1.0)
    nc.vector.memset(epst[:].bitcast(F32), 1e-5)
    nc.gpsimd.memset(vloc[:, :, :, HD:HD + 1], 1.0)
    nc.gpsimd.memset(vpe[:, :, :, HD:HD + 1], 1.0)
    nc.sync.dma_start(out=mt[:], in_=masks_d.rearrange("t p n -> p t n"))
    nc.sync.dma_start(out=xsT[:], in_=xsT_d[:])
    nc.sync.dma_start(out=rw[:], in_=rw_d[:])
    wot = persist.tile([128, DC, VOCAB], BF16)
    nc.sync.dma_start(out=wot[:], in_=wo_d.rearrange("(c p) v -> p c v", p=128))

    def new_stats(nm):
        """PSUM accumulators for the next LN; e2 group opens with the eps term."""
        mu_ps = ps_misc.tile([1, TLOC], F32, tag="pm", name=f"mu_{nm}")
        e2_ps = ps_misc.tile([1, TLOC], F32, tag="pm", name=f"e2_{nm}")
        nc.tensor.matmul(e2_ps[:], epst[:], onesr[:], start=True, stop=False)
        return mu_ps, e2_ps

    def res_square(oc):
        """x2s[oc] = h[oc]^2 on the scalar engine (Square is in every table)."""
        nc.scalar.activation(out=x2s[:, oc, :], in_=h[:, oc, :],
                             func=AF.Square, bias=0.0)

    def stats_mms(stats):
        mu_ps, e2_ps = stats
        for c in range(DC):
            nc.tensor.matmul(mu_ps[:], onesc[:], h[:, c, :],
                             start=(c == 0), stop=(c == DC - 1))
        for c in range(DC):
            nc.tensor.matmul(e2_ps[:], onesc[:], x2s[:, c, :],
                             start=False, stop=(c == DC - 1))

    def ln_normalize(stats):
        """consume stats -> xhat (bf16, pure-normalized)"""
        mu_ps, e2_ps = stats
        mu = small.tile([1, TLOC], F32R, tag="sm")
        var = small.tile([1, TLOC], F32, tag="sm")
        rr = small.tile([1, TLOC], F32R, tag="sm")
        y = small.tile([1, TLOC], F32, tag="sm")
        t = small.tile([1, TLOC], F32, tag="sm")
        nc.vector.tensor_copy(mu[:], mu_ps[:])
        nc.vector.tensor_mul(var[:], mu[:], mu[:])
        nc.vector.tensor_sub(var[:], e2_ps[:], var[:])
        # rstd via bit-trick + 1 Newton step (keeps LN off the act tables)
        nc.vector.tensor_scalar(y[:].bitcast(I32), var[:].bitcast(I32), 1, None,
                                Alu.logical_shift_right)
        nc.vector.tensor_scalar(y[:].bitcast(I32), y[:].bitcast(I32),
                                RSQRT_MAGIC, -1, Alu.subtract, Alu.mult)
        nc.vector.tensor_mul(t[:], y[:], y[:])
        nc.vector.tensor_mul(t[:], t[:], var[:])
        nc.vector.tensor_scalar(t[:], t[:], -0.5, 1.5, Alu.mult, Alu.add)
        with nc.allow_low_precision(reason="f32r rstd is plenty for LN"):
            nc.vector.tensor_mul(rr[:], y[:], t[:])
        mub_ps = ps_misc.tile([128, TLOC], F32, tag="pm")
        rsb_ps = ps_misc.tile([128, TLOC], F32, tag="pm")
        nc.tensor.matmul(mub_ps[:], onesr[0:1, 0:128], mu[:],
                         start=True, stop=True)
        nc.tensor.matmul(rsb_ps[:], onesr[0:1, 0:128], rr[:],
                         start=True, stop=True)
        xs_ = scr.tile([128, DC, TLOC], F32, tag="s8c")
        for c in range(DC):
            nc.vector.tensor_sub(xs_[:, c, :], h[:, c, :], mub_ps[:])
        for c in range(DC):
            nc.vector.tensor_mul(xhat[:, c, :], xs_[:, c, :], rsb_ps[:])

    eng = nc.gpsimd
    pid = eng.partition_id()
    rpar = eng.alloc_register("rpar")
    eng.reg_mod(rpar, pid, 2)
    rpeer = eng.alloc_register("rpeer")
    eng.reg_alu(rpeer, 1, rpar, mybir.AluOpType.subtract)
    rxb = eng.alloc_register("rxb")
    eng.reg_mul(rxb, rpeer, X_BF16)
    xbase_sv = eng.snap(rxb, donate=True, min_val=0, max_val=X_BF16)

    def emit_wdma(i):
        """Queue the bf16 weight loads for layer i (ring-buffered pool)."""
        tiles = {}
        tiles["w1k"] = wpool.tile([128, DC, D], BF16, tag="w", name=f"w1k_{i}")
        nc.sync.dma_start(out=tiles["w1k"][:], in_=w1_d[i, :, D:2 * D]
                          .rearrange("(c p) o -> p c o", p=128))
        tiles["w1q"] = wpool.tile([128, DC, D], BF16, tag="w", name=f"w1q_{i}")
        nc.sync.dma_start(out=tiles["w1q"][:], in_=w1_d[i, :, 0:D]
                          .rearrange("(c p) o -> p c o", p=128))
        tiles["w1v"] = wpool.tile([128, DC, D], BF16, tag="w", name=f"w1v_{i}")
        nc.sync.dma_start(out=tiles["w1v"][:], in_=w1_d[i, :, 2 * D:3 * D]
                          .rearrange("(c p) o -> p c o", p=128))
        tiles["wp"] = wpool.tile([128, DC, D], BF16, tag="w", name=f"wp_{i}")
        nc.sync.dma_start(out=tiles["wp"][:], in_=wp_d[i]
                          .rearrange("(c p) o -> p c o", p=128))
        tiles["w2"] = []
        for qi in range(4):
            w2t = wpool.tile([128, DC, D], BF16, tag="w", name=f"w2_{i}_{qi}")
            nc.sync.dma_start(out=w2t[:], in_=w2_d[i, :, qi * D:(qi + 1) * D]
                              .rearrange("(c p) o -> p c o", p=128))
            tiles["w2"].append(w2t)
        tiles["w3"] = []
        for qi in range(4):
            w3t = wpool.tile([128, DC, D], BF16, tag="w", name=f"w3_{i}_{qi}")
            nc.sync.dma_start(out=w3t[:], in_=w3_d[i, qi * D:(qi + 1) * D, :]
                              .rearrange("(c p) o -> p c o", p=128))
            tiles["w3"].append(w3t)
        return tiles

    # ---- embed: h = read_w.T @ xsT + posT, with LN1 stats chasing ----
    posTt = scr.tile([128, DC, TLOC], F32, tag="s8b")
    nc.sync.dma_start(out=posTt[:], in_=posT_d.rearrange("c p t -> p c t"))
    wtiles = emit_wdma(0 % L)
    stats = new_stats("embed")
    for oc in range(DC):
        ps = ps_mm.tile([128, TLOC], F32, tag="mm")
        nc.tensor.matmul(ps[:], rw[0:NDIMS, oc * 128:(oc + 1) * 128], xsT[:],
                         start=True, stop=True)
        nc.vector.tensor_add(h[:, oc, :], ps[:], posTt[:, oc, :])
        res_square(oc)
    stats_mms(stats)

    for li in range(n_layers):
        i = li % L
        lt = 0 if i < 2 else 1
        w1k, w1v, w1q = wtiles["w1k"], wtiles["w1v"], wtiles["w1q"]
        wpt, w2q, w3q = wtiles["wp"], wtiles["w2"], wtiles["w3"]

        # ---- LN1, then exchange xhat with the pair peer ----
        with nc.named_scope(f"ln1_{li}"):
            ln_normalize(stats)
        nc.sync.dma_start(
            out=bounce[:].bitcast(BF16).rearrange("(p n) -> p n", p=128),
            in_=xhat[:].rearrange("p c t -> p (c t)"))
        if fake_ag:
            nc.sync.dma_start(out=agout[0:X_WORDS], in_=bounce[:])
            nc.sync.dma_start(out=agout[X_WORDS:2 * X_WORDS], in_=bounce[:])
        else:
            nc.gpsimd.collective_compute(
                "AllGather", mybir.AluOpType.bypass, replica_groups=PAIRS,
                ins=[bounce[:]], outs=[agout[:]])
        nc.gpsimd.dma_start(
            out=xrem[:].rearrange("p c t -> p (c t)"),
            in_=agout[:].bitcast(BF16)[bass.ds(xbase_sv, X_BF16)]
                .rearrange("(p n) -> p n", p=128))
        # weight prefetch for the next layer rides behind the xhat DMA
        if li + 1 < n_layers:
            next_tiles = emit_wdma((li + 1) % L)

        # ---- local QKV projections (overlap the collective) ----
        for oc in range(DC):   # k and q interleaved, feature-major
            ps = ps_mm.tile([128, TLOC], F32, tag="mm")
            for c in range(DC):
                nc.tensor.matmul(ps[:], w1k[:, c, oc * 128:(oc + 1) * 128],
                                 xhat[:, c, :], start=(c == 0),
                                 stop=(c == DC - 1))
            nc.scalar.copy(out=kst[:, oc, :], in_=ps[:])
            ps = ps_mm.tile([128, TLOC], F32, tag="mm")
            for c in range(DC):
                nc.tensor.matmul(ps[:], w1q[:, c, oc * 128:(oc + 1) * 128],
                                 xhat[:, c, :], start=(c == 0),
                                 stop=(c == DC - 1))
            nc.scalar.copy(out=qT[:, oc, :], in_=ps[:])
        for tcb in range(4):   # v, token-major: vT = xhat.T @ Wv
            ps = ps_mm.tile([128, TLOC], F32, tag="mm")
            for c in range(DC):
                nc.tensor.matmul(ps[:], xhat[:, c, tcb * 128:(tcb + 1) * 128],
                                 w1v[:, c, :], start=(c == 0),
                                 stop=(c == DC - 1))
            nc.vector.tensor_copy(
                vloc[:, tcb, :, 0:HD], ps[:].rearrange("p (h d) -> p h d", h=H))

        # ---- attention pass 1: local key blocks, all heads ----
        for hh in range(H):
            hc, hr = hh // 2, (hh % 2) * HD
            cps = ps_cps.tile([HD + 1, TLOC], F32, tag="cps")
            for vi in range(4):
                s, w = vi, NPAD_V[vi]
                sps = ps_mm.tile([128, TLOC], F32, tag="mm")
                nc.tensor.matmul(sps[:, 0:w],
                                 kst[hr:hr + HD, hc, s * 128:(s + 1) * 128],
                                 qT[hr:hr + HD, hc, TLOC - w:TLOC],
                                 start=True, stop=True)
                pt = ppool.tile([128, TLOC], BF16, tag="P")
                nc.scalar.activation(out=pt[:, 0:w], in_=sps[:, 0:w],
                                     func=AF.Exp, scale=0.125)
                mw = w if (lt == 0 and s == 0) else 128
                nc.gpsimd.tensor_mul(pt[:, 0:mw], pt[:, 0:mw],
                                     mt[:, lt, OFF[vi]:OFF[vi] + mw])
                nc.tensor.matmul(cps[:, TLOC - w:TLOC], vloc[:, s, hh, :],
                                 pt[:, 0:w], start=(vi == 0), stop=(vi == 3))
            nc.vector.tensor_copy(ctxl[:, hh, :], cps[:])

        # ---- peer K/V projections from the gathered xhat ----
        for oc in range(DC):
            ps = ps_mm.tile([128, TLOC], F32, tag="mm")
            for c in range(DC):
                nc.tensor.matmul(ps[:], w1k[:, c, oc * 128:(oc + 1) * 128],
                                 xrem[:, c, :], start=(c == 0),
                                 stop=(c == DC - 1))
            nc.scalar.copy(out=kpe[:, oc, :], in_=ps[:])
        for tcb in range(4):
            ps = ps_mm.tile([128, TLOC], F32, tag="mm")
            for c in range(DC):
                nc.tensor.matmul(ps[:], xrem[:, c, tcb * 128:(tcb + 1) * 128],
                                 w1v[:, c, :], start=(c == 0),
                                 stop=(c == DC - 1))
            nc.vector.tensor_copy(
                vpe[:, tcb, :, 0:HD], ps[:].rearrange("p (h d) -> p h d", h=H))

        # ---- attention pass 2: remote key blocks + normalize ----
        for hh in range(H):
            hc, hr = hh // 2, (hh % 2) * HD
            cps = ps_cps.tile([HD + 1, TLOC], F32, tag="cps")
            for vi in range(4, NB):
                s, w = vi % 4, NPAD_V[vi]
                sps = ps_mm.tile([128, TLOC], F32, tag="mm")
                nc.tensor.matmul(sps[:, 0:w],
                                 kpe[hr:hr + HD, hc, s * 128:(s + 1) * 128],
                                 qT[hr:hr + HD, hc, TLOC - w:TLOC],
                                 start=True, stop=True)
                pt = ppool.tile([128, TLOC], BF16, tag="P")
                nc.scalar.activation(out=pt[:, 0:w], in_=sps[:, 0:w],
                                     func=AF.Exp, scale=0.125)
                mw = w if (lt == 0 and s == 0) else 128
                nc.gpsimd.tensor_mul(pt[:, 0:mw], pt[:, 0:mw],
                                     mt[:, lt, OFF[vi]:OFF[vi] + mw])
                nc.tensor.matmul(cps[:, TLOC - w:TLOC], vpe[:, s, hh, :],
                                 pt[:, 0:w], start=(vi == 4), stop=(vi == 7))
            nc.vector.tensor_add(ctxl[:, hh, :], ctxl[:, hh, :], cps[:])
            rec = small.tile([1, TLOC], F32R, tag="sm")
            with nc.allow_low_precision(reason="f32r softmax denom recip"):
                nc.vector.reciprocal(rec[:], ctxl[HD:HD + 1, hh, :])
            rb = ps_misc.tile([HD, TLOC], F32, tag="pm", name=f"rb{hh}")
            nc.tensor.matmul(rb[:], onesr[0:1, 0:HD], rec[:], start=True, stop=True)
            nc.vector.tensor_mul(ctxf[hr:hr + HD, hc, :], ctxl[0:HD, hh, :], rb[:])
        # preload the gelu table while the out-projection runs
        nc.scalar.activation(out=scr1[:], in_=epst[:].bitcast(F32),
                             func=AF.Gelu_apprx_tanh, bias=0.0)

        # ---- attention out-projection + residual, LN2 stats chasing ----
        stats = new_stats(f"l{li}a")
        for oc in range(DC):
            ps = ps_mm.tile([128, TLOC], F32, tag="mm")
            for c in range(DC):
                nc.tensor.matmul(ps[:], wpt[:, c, oc * 128:(oc + 1) * 128],
                                 ctxf[:, c, :], start=(c == 0),
                                 stop=(c == DC - 1))
            nc.vector.tensor_add(h[:, oc, :], h[:, oc, :], ps[:])
            res_square(oc)
        stats_mms(stats)

        # ---- LN2 + MLP ----
        with nc.named_scope(f"ln2_{li}"):
            ln_normalize(stats)
        for oc in range(16):
            ps = ps_mm.tile([128, TLOC], F32, tag="mm")
            for c in range(DC):
                nc.tensor.matmul(ps[:], w2q[oc // 4][:, c, (oc % 4) * 128:(oc % 4 + 1) * 128],
                                 xhat[:, c, :], start=(c == 0),
                                 stop=(c == DC - 1))
            nc.scalar.activation(out=gel[:, oc, :], in_=ps[:],
                                 func=AF.Gelu_apprx_tanh, bias=0.0)
        # preload the exp table while the down-projection runs
        nc.scalar.activation(out=scr1[:], in_=epst[:].bitcast(F32),
                             func=AF.Exp, scale=1.0)
        stats = new_stats(f"l{li}m")
        for oc in range(DC):
            pp = ps_mm.tile([128, TLOC], F32, tag="mm")
            for kc in range(16):
                nc.tensor.matmul(pp[:], w3q[kc // 4][:, kc % 4, oc * 128:(oc + 1) * 128],
                                 gel[:, kc, :], start=(kc == 0),
                                 stop=(kc == 15))
            nc.vector.tensor_add(h[:, oc, :], h[:, oc, :], pp[:])
            res_square(oc)
        stats_mms(stats)
        if li + 1 < n_layers:
            wtiles = next_tiles

    # ---- final LN + vocab projection ----
    ln_normalize(stats)
    ps = ps_mm.tile([VOCAB, TLOC], F32, tag="mm")
    for c in range(DC):
        nc.tensor.matmul(ps[:], wot[:, c, :], xhat[:, c, :],
                         start=(c == 0), stop=(c == DC - 1))
    osb = opool.tile([VOCAB, TLOC], F32, tag="osb")
    nc.scalar.copy(out=osb[:], in_=ps[:])
    nc.sync.dma_start(out=out_d[:], in_=osb[:])
    ctx.close()


def _valid_full():
    """valid[lt, k, q] over global token ids."""
    q = np.arange(S)[None, :]
    k = np.arange(S)[:, None]
    causal = k <= q
    # layer type 0 (mask_first)
    blk = (k // 4 == q // 4) & (q < 20) & (k < 20)
    row20 = (q == 20) & (k <= 20)
    path0 = (q >= SCHEMA) & (k >= SCHEMA)
    m0 = (blk | row20 | path0) & causal
    return np.stack([m0, causal])


def _prep(inputs):
    f32 = lambda a: np.ascontiguousarray(np.asarray(a), dtype=np.float32)
    bf = ml_dtypes.bfloat16
    xs = f32(inputs["xs"])
    read_w, read_b = f32(inputs["read_w"]), f32(inputs["read_b"])
    pos = np.concatenate([f32(inputs["pos_schema"]),
                          f32(inputs["pos_path"])[: S - SCHEMA]], axis=0)
    ln1_g = f32(inputs["ln1_g"])
    ln2_g = f32(inputs["ln2_g"])
    lnf_g = f32(inputs["lnf_g"])
    attn_w = f32(inputs["attn_w"])
    attnp_w = f32(inputs["attnp_w"])
    fc_w = f32(inputs["fc_w"])
    proj_w = f32(inputs["proj_w"])
    out_w = f32(inputs["out_w"])

    # biases are all zero for this model; LN gains fold into the weights
    ln1_b, attn_b = f32(inputs["ln1_b"]), f32(inputs["attn_b"])
    ln2_b, fc_b = f32(inputs["ln2_b"]), f32(inputs["fc_b"])
    lnf_b = f32(inputs["lnf_b"])
    b1 = np.einsum("ld,ldo->lo", ln1_b, attn_w) + attn_b
    b2 = np.einsum("ld,ldo->lo", ln2_b, fc_w) + fc_b
    bo = lnf_b @ out_w + f32(inputs["out_b"])
    assert not any(np.any(v) for v in
                   (b1, f32(inputs["attnp_b"]), b2, f32(inputs["proj_b"]), bo)), \
        "bias-free fast path: nonzero biases not supported"

    w1 = (attn_w * ln1_g[:, :, None]).astype(bf)
    w2 = (fc_w * ln2_g[:, :, None]).astype(bf)
    wo = (out_w * lnf_g[:, None]).astype(bf)
    wp = attnp_w.astype(bf)
    w3 = proj_w.astype(bf)

    valid = _valid_full()
    shared = dict(rw=read_w.astype(np.float32), w1=w1, wp=wp,
                  w2=w2, w3=w3, wo=wo)

    in_maps = []
    for c in range(8):
        b = c // 2
        blocks = H0_BLOCKS if c % 2 == 0 else H1_BLOCKS
        toks = np.concatenate([np.arange(bb * TB, (bb + 1) * TB) for bb in blocks])
        xsT = np.ascontiguousarray(xs[b][toks].T)                    # (64, 512)
        posT = (pos[toks] + read_b[None, :]).T                        # (512, 512)
        posT = np.ascontiguousarray(posT.reshape(DC, 128, TLOC))
        peer_blocks = H1_BLOCKS if c % 2 == 0 else H0_BLOCKS
        vslot_blocks = list(blocks) + list(peer_blocks)
        masks = np.zeros((2, 128, SUM_NPAD), dtype=bf)
        for lt in range(2):
            for vi, j in enumerate(vslot_blocks):
                w = NPAD_V[vi]
                cols = toks[TLOC - w:]
                masks[lt, :, OFF[vi]:OFF[vi] + w] = \
                    valid[lt, j * TB:(j + 1) * TB][:, cols].astype(bf)
        m = dict(shared)
        m.update(xsT=xsT, posT=posT, masks=masks)
        in_maps.append(m)
    return in_maps


def kernel(**inputs):
    global LAST_RESULTS
    in_maps = _prep(inputs)
    key = L
    if key not in _PROGRAM_CACHE:
        _PROGRAM_CACHE[key] = _build_program(L)
    nc = _PROGRAM_CACHE[key]
    bench = int(os.environ.get("KBENCH_REPS", "0"))
    results = _run_spmd(nc, in_maps, bench_reps=bench)
    LAST_RESULTS = results

    out = np.zeros((4, S, VOCAB), dtype=np.float32)
    for c in range(8):
        b = c // 2
        blocks = H0_BLOCKS if c % 2 == 0 else H1_BLOCKS
        o = results[c]["outT"]                                        # (19, 512)
        for bi, bb in enumerate(blocks):
            out[b, bb * TB:(bb + 1) * TB, :] = o[:, bi * TB:(bi + 1) * TB].T
    return out


# revision 37
# speedup vs baseline: 1.1069x; 1.1069x over previous
"""Trainium2 Bass kernel for a 12-layer autoregressive transformer.

Sharding: 4 batch elements x 2-way sequence split across 8 cores.
Core pair p = (2p, 2p+1) handles batch element p. Within a pair, core
half 0 owns 128-token blocks [0,3,4,7], half 1 owns [1,2,5,6] (this
balances causal-attention work exactly: 18 block-pairs each).

Per layer, the two cores of a pair exchange the LN1 output xhat
(bf16, 512KB) with one AllGather launched right after LN1; each core
then computes the peer's K/V projections locally from the gathered
xhat (cheaper than exchanging K+V, and the collective overlaps the
local QKV projections + the local half of attention). Attention runs
in two passes: local key blocks for all heads first (no dependency on
the collective), then remote key blocks.

On-device layout is feature-major (features on SBUF partitions, tokens
on the free axis). All weights are bf16 (same PE rate as f32r, half
the HBM traffic), prefetched one layer ahead. The residual stream h
stays fp32. LN statistics are computed via PE column-sum matmuls; the
per-token 1/sqrt(var) uses a DVE bit-trick + 2 Newton steps so the
scalar engine's activation table never swaps for LN (only exp<->gelu
swaps remain). Softmax denominators come from a ones-column appended
to V; per-token (free-axis) broadcasts are K=1 matmuls on the PE.
"""

import os
import numpy as np
import ml_dtypes

import concourse.bass as bass
import concourse.mybir as mybir
import concourse.tile as tile
from concourse import bacc
from concourse.bass_utils import run_bass_kernel_spmd

F32 = mybir.dt.float32
F32R = mybir.dt.float32r
BF16 = mybir.dt.bfloat16
I32 = mybir.dt.int32
I8 = mybir.dt.int8
QDT = None  # set in _build_program

S, D, H, HD, L, DFF, VOCAB = 1024, 512, 8, 64, 12, 2048, 19
SCHEMA, NDIMS = 21, 64
NB, TB = 8, 128            # token blocks of 128
TLOC = 512                 # tokens per core
DC = D // 128              # 4 feature chunks
H0_BLOCKS = [0, 3, 4, 7]
H1_BLOCKS = [1, 2, 5, 6]
# padded q-window widths per key block (max over the two halves' suffix counts)
# virtual attention slots: 4 local blocks then 4 remote (peer) blocks, each
# ordered ascending; q-window width for slot s is (4 - s) * 128 padded to the
# max over halves -- identical for both halves by construction of the split.
NPAD_V = [512, 384, 256, 128, 512, 384, 256, 128]
OFF = np.concatenate([[0], np.cumsum(NPAD_V)]).astype(int)
SUM_NPAD = int(OFF[-1])                # 2816

X_ELEMS = DC * 128 * TLOC              # xhat half, elems (262144)
QDT_BYTES = int(os.environ.get("KQ_BYTES", "1"))   # 1=int8, 2=bf16 exchange
X_WORDS = X_ELEMS * QDT_BYTES // 4     # f32r words in bounce
PAIRS = [[0, 1], [2, 3], [4, 5], [6, 7]]
RSQRT_MAGIC = 0x5F3759DF
QSCALE = 20.0
KDEFER = os.environ.get("KDEFER", "1") == "1"

_PROGRAM_CACHE = {}
LAST_RESULTS = None
LAST_EXEC_S = None


def _run_spmd(nc, in_maps, n_cores=8, bench_reps=0):
    """Execute a prebuilt Bass module on 8 cores via PJRT (axon), jitting
    once; optionally re-run the warm executable to measure execution time."""
    global LAST_EXEC_S
    import time
    import jax
    from jax.experimental.shard_map import shard_map
    from jax.sharding import Mesh, PartitionSpec
    from concourse import bass2jax, mybir as _mybir
    bass2jax.install_neuronx_cc_hook()

    partition_name = nc.partition_id_tensor.name if nc.partition_id_tensor else None
    in_names, out_names, out_avals, zero_outs = [], [], [], []
    for alloc in nc.m.functions[0].allocations:
        if not isinstance(alloc, _mybir.MemoryLocationSet):
            continue
        name = alloc.memorylocations[0].name
        if alloc.kind == "ExternalInput":
            if name != partition_name:
                in_names.append(name)
        elif alloc.kind == "ExternalOutput":
            shape = tuple(alloc.tensor_shape)
            dtype = _mybir.dt.np(alloc.dtype)
            out_names.append(name)
            out_avals.append(jax.core.ShapedArray(shape, dtype))
            zero_outs.append(np.zeros(shape, dtype))
    n_params = len(in_names)
    n_outs = len(out_avals)
    all_in_names = list(in_names) + list(out_names)
    if partition_name is not None:
        all_in_names.append(partition_name)

    def _body(*args):
        operands = list(args)
        if partition_name is not None:
            operands.append(bass2jax.partition_id_tensor())
        outs = bass2jax._bass_exec_p.bind(
            *operands, out_avals=tuple(out_avals), in_names=tuple(all_in_names),
            out_names=tuple(out_names), lowering_input_output_aliases=(),
            sim_require_finite=True, sim_require_nnan=True, nc=nc)
        return tuple(outs)

    devices = jax.devices()[:n_cores]
    mesh = Mesh(np.asarray(devices), ("core",))
    in_specs = (PartitionSpec("core"),) * (n_params + n_outs)
    out_specs = (PartitionSpec("core"),) * n_outs
    donate = tuple(range(n_params, n_params + n_outs))
    sharded = jax.jit(
        shard_map(_body, mesh=mesh, in_specs=in_specs, out_specs=out_specs,
                  check_rep=False),
        donate_argnums=donate, keep_unused=True)

    concat_in = [np.concatenate([np.asarray(in_maps[c][nm])[None]
                                 for c in range(n_cores)], axis=0)
                 .reshape(n_cores * np.asarray(in_maps[0][nm]).shape[0],
                          *np.asarray(in_maps[0][nm]).shape[1:])
                 for nm in in_names]
    def _zeros():
        return [np.zeros((n_cores * z.shape[0], *z.shape[1:]), z.dtype)
                for z in zero_outs]

    out_arrs = jax.block_until_ready(sharded(*concat_in, *_zeros()))

    if bench_reps:
        from jax.sharding import NamedSharding
        shardings = [NamedSharding(mesh, PartitionSpec("core"))] * len(concat_in)
        dev_in = [jax.device_put(a, s) for a, s in zip(concat_in, shardings)]
        jax.block_until_ready(dev_in)
        times = []
        for _ in range(bench_reps):
            zo = [jax.device_put(z, NamedSharding(mesh, PartitionSpec("core")))
                  for z in _zeros()]
            jax.block_until_ready(zo)
            t0 = time.perf_counter()
            r = jax.block_until_ready(sharded(*dev_in, *zo))
            times.append(time.perf_counter() - t0)
        LAST_EXEC_S = min(times)

    return [{nm: np.asarray(out_arrs[i]).reshape(n_cores, *out_avals[i].shape)[c]
             for i, nm in enumerate(out_names)} for c in range(n_cores)]


def _build_program(n_layers=L, fake_ag=False):
    global QDT
    QDT = I8 if QDT_BYTES == 1 else BF16
    nc = bacc.Bacc("TRN2", target_bir_lowering=False, num_devices=8)

    # ---------------- DRAM I/O ----------------
    xsT_d = nc.dram_tensor("xsT", [NDIMS, TLOC], F32R, kind="ExternalInput")
    posT_d = nc.dram_tensor("posT", [DC, 128, TLOC], F32, kind="ExternalInput")
    masks_d = nc.dram_tensor("masks", [2, 128, SUM_NPAD], BF16, kind="ExternalInput")
    rw_d = nc.dram_tensor("rw", [NDIMS, D], F32R, kind="ExternalInput")
    w1_d = nc.dram_tensor("w1", [L, D, 3 * D], BF16, kind="ExternalInput")
    wp_d = nc.dram_tensor("wp", [L, D, D], BF16, kind="ExternalInput")
    w2_d = nc.dram_tensor("w2", [L, D, DFF], BF16, kind="ExternalInput")
    w3_d = nc.dram_tensor("w3", [L, DFF, D], BF16, kind="ExternalInput")
    wo_d = nc.dram_tensor("wo", [D, VOCAB], BF16, kind="ExternalInput")
    out_d = nc.dram_tensor("outT", [VOCAB, TLOC], F32, kind="ExternalOutput")

    bounce = nc.dram_tensor("bounce", [2, X_WORDS], F32R)
    rrd = nc.dram_tensor("rrd", [TLOC], F32)
    dbg_xrem = dbg_xrb = None
    if os.environ.get("KDEBUG"):
        dbg_xrem = nc.dram_tensor("dbg_xrem", [128, DC * TLOC], mybir.dt.int8 if QDT_BYTES == 1 else mybir.dt.bfloat16, kind="ExternalOutput")
        dbg_xrb = nc.dram_tensor("dbg_xrb", [128, DC * TLOC], mybir.dt.bfloat16, kind="ExternalOutput")
        d_kpe = nc.dram_tensor("d_kpe", [128, DC * TLOC], mybir.dt.bfloat16, kind="ExternalOutput")
        d_vpe = nc.dram_tensor("d_vpe", [128, 4 * H * (HD + 1)], mybir.dt.bfloat16, kind="ExternalOutput")
        d_kst = nc.dram_tensor("d_kst", [128, DC * TLOC], mybir.dt.bfloat16, kind="ExternalOutput")
        d_qT = nc.dram_tensor("d_qT", [128, DC * TLOC], mybir.dt.bfloat16, kind="ExternalOutput")
        d_vloc = nc.dram_tensor("d_vloc", [128, 4 * H * (HD + 1)], mybir.dt.bfloat16, kind="ExternalOutput")
        d_ctxf = nc.dram_tensor("d_ctxf", [128, DC * TLOC], mybir.dt.bfloat16, kind="ExternalOutput")
        d_xq = nc.dram_tensor("d_xq", [128, DC * TLOC], mybir.dt.int8 if QDT_BYTES == 1 else mybir.dt.bfloat16, kind="ExternalOutput")
    agout = nc.dram_tensor("agout", [2, 2 * X_WORDS], F32R)

    with tile.TileContext(nc) as tc:
        _emit(nc, tc, locals(), n_layers, fake_ag)
    nc.compile()
    return nc


def _emit(nc, tc, d, n_layers, fake_ag=False):
    xsT_d, posT_d, masks_d, rw_d = d["xsT_d"], d["posT_d"], d["masks_d"], d["rw_d"]
    w1_d, wp_d, w2_d, w3_d, wo_d = d["w1_d"], d["wp_d"], d["w2_d"], d["w3_d"], d["wo_d"]
    out_d, bounce, agout = d["out_d"], d["bounce"], d["agout"]
    rrd = d["rrd"]
    dbg_xrem, dbg_xrb = d["dbg_xrem"], d["dbg_xrb"]
    d_kpe, d_vpe = d.get("d_kpe"), d.get("d_vpe")
    d_kst, d_qT, d_vloc, d_ctxf = d.get("d_kst"), d.get("d_qT"), d.get("d_vloc"), d.get("d_ctxf")
    d_xq = d.get("d_xq")
    AF = mybir.ActivationFunctionType
    Alu = mybir.AluOpType

    import contextlib
    ctx = contextlib.ExitStack()
    persist = ctx.enter_context(tc.tile_pool(name="persist", bufs=1))
    scr = ctx.enter_context(tc.tile_pool(name="scr", bufs=1))
    wpool = ctx.enter_context(tc.tile_pool(name="wpool", bufs=14))
    ppool = ctx.enter_context(tc.tile_pool(name="ppool", bufs=10))
    small = ctx.enter_context(tc.tile_pool(name="small", bufs=4))
    opool = ctx.enter_context(tc.tile_pool(name="opool", bufs=1))
    ps_mm = ctx.enter_context(tc.tile_pool(name="ps_mm", bufs=4, space="PSUM"))
    ps_cps = ctx.enter_context(tc.tile_pool(name="ps_cps", bufs=2, space="PSUM"))
    ps_misc = ctx.enter_context(tc.tile_pool(name="ps_misc", bufs=2, space="PSUM"))

    # ---- persistent tiles ----
    h = persist.tile([128, DC, TLOC], F32R)
    xhat = persist.tile([128, DC, TLOC], BF16)
    xhc = [persist.tile([128, TLOC], BF16, name=f"xhc{c}") for c in range(DC)]
    xrem = persist.tile([128, DC, TLOC], QDT)
    xrb = [persist.tile([128, TLOC], BF16, name=f"xrb{c}") for c in range(DC)]
    xq = persist.tile([128, DC, TLOC], QDT)
    rsb_sb = persist.tile([128, TLOC], F32)
    rcols = persist.tile([128, 4], F32)
    qT = persist.tile([128, DC, TLOC], BF16)
    kst = persist.tile([128, DC, TLOC], BF16)
    kpe = persist.tile([128, DC, TLOC], BF16)
    vloc = persist.tile([128, 4, H, HD + 1], BF16)
    vpe = persist.tile([128, 4, H, HD + 1], BF16)
    ctxl = persist.tile([HD + 1, H, TLOC], F32)
    ctxf = persist.tile([128, DC, TLOC], BF16)
    mt = persist.tile([128, 2, SUM_NPAD], BF16)
    gel = persist.tile([128, 16, TLOC], BF16)
    x2s = persist.tile([128, DC, TLOC], F32R)   # squares for LN stats
    onesc = persist.tile([128, 1], F32R)        # 1/512 column (mean via matmul)
    onesr = persist.tile([1, TLOC], F32R)       # exact ones row
    epst = persist.tile([1, 1], F32R)           # eps value for var
    scr1 = persist.tile([1, 1], F32)            # scratch for act-table preloads
    xsT = persist.tile([NDIMS, TLOC], F32R)
    rw = persist.tile([NDIMS, D], F32R)

    nc.vector.memset(onesc[:].bitcast(F32), 1.0 / D)
    nc.vector.memset(onesr[:].bitcast(F32), 1.0)
    nc.vector.memset(epst[:].bitcast(F32), 1e-5)
    nc.gpsimd.memset(vloc[:, :, :, HD:HD + 1], 1.0)
    nc.gpsimd.memset(vpe[:, :, :, HD:HD + 1], 1.0)
    nc.sync.dma_start(out=mt[:], in_=masks_d.rearrange("t p n -> p t n"))
    nc.sync.dma_start(out=xsT[:], in_=xsT_d[:])
    nc.sync.dma_start(out=rw[:], in_=rw_d[:])
    wot = persist.tile([128, DC, VOCAB], BF16)
    nc.sync.dma_start(out=wot[:], in_=wo_d.rearrange("(c p) v -> p c v", p=128))

    def new_stats(nm):
        """PSUM accumulators for the next LN; e2 group opens with the eps term."""
        mu_ps = ps_misc.tile([1, TLOC], F32, tag="pm", name=f"mu_{nm}")
        e2_ps = ps_misc.tile([1, TLOC], F32, tag="pm", name=f"e2_{nm}")
        nc.tensor.matmul(e2_ps[:], epst[:], onesr[:], start=True, stop=False)
        return mu_ps, e2_ps

    def res_square(oc):
        """x2s[oc] = h[oc]^2 on the scalar engine (Square is in every table)."""
        nc.scalar.activation(out=x2s[:, oc, :], in_=h[:, oc, :],
                             func=AF.Square, bias=0.0)

    def stats_mms(stats):
        mu_ps, e2_ps = stats
        for c in range(DC):
            nc.tensor.matmul(mu_ps[:], onesc[:], h[:, c, :],
                             start=(c == 0), stop=(c == DC - 1))
        for c in range(DC):
            nc.tensor.matmul(e2_ps[:], onesc[:], x2s[:, c, :],
                             start=False, stop=(c == DC - 1))

    def _rstd_chain(var, rr, newton=1):
        """rr = 1/sqrt(var) via bit-trick + Newton (no act-table traffic)."""
        y = small.tile([1, TLOC], F32, tag="sm")
        t = small.tile([1, TLOC], F32, tag="sm")
        nc.vector.tensor_scalar(y[:].bitcast(I32), var[:].bitcast(I32), 1, None,
                                Alu.logical_shift_right)
        nc.vector.tensor_scalar(y[:].bitcast(I32), y[:].bitcast(I32),
                                RSQRT_MAGIC, -1, Alu.subtract, Alu.mult)
        for it in range(newton):
            nc.vector.tensor_mul(t[:], y[:], y[:])
            nc.vector.tensor_mul(t[:], t[:], var[:])
            nc.vector.tensor_scalar(t[:], t[:], -0.5, 1.5, Alu.mult, Alu.add)
            if it + 1 < newton:
                nc.vector.tensor_mul(y[:], y[:], t[:])
        with nc.allow_low_precision(reason="f32r rstd is plenty for LN"):
            nc.vector.tensor_mul(rr[:], y[:], t[:])

    def ln_defer(stats, parity=0, want_cols=True, want_quant=True):
        """consume stats -> xhc holds UNSCALED (h - mu) bf16; rstd lands in
        rsb_ps/rsb_sb (row broadcast) and rcols (per-token-block columns);
        if want_quant, xq gets int8-quantized normalized xhat for exchange,
        emitted chunk-wise so the bounce DMAs chase the subs."""
        mu_ps, e2_ps = stats
        mu = small.tile([1, TLOC], F32R, tag="sm")
        var = small.tile([1, TLOC], F32, tag="sm")
        rr = small.tile([1, TLOC], F32R, tag="sm")
        nc.vector.tensor_copy(mu[:], mu_ps[:])
        nc.vector.tensor_mul(var[:], mu[:], mu[:])
        nc.vector.tensor_sub(var[:], e2_ps[:], var[:])
        _rstd_chain(var, rr)
        rsb_ps = ps_misc.tile([128, TLOC], F32, tag="pm")
        nc.tensor.matmul(rsb_ps[:], onesr[0:1, 0:128], rr[:],
                         start=True, stop=True)
        mub_ps = ps_misc.tile([128, TLOC], F32, tag="pm")
        nc.tensor.matmul(mub_ps[:], onesr[0:1, 0:128], mu[:],
                         start=True, stop=True)
        nc.vector.tensor_copy(rsb_sb[:], rsb_ps[:])
        for c in range(DC):
            nc.vector.tensor_sub(xhc[c][:], h[:, c, :], mub_ps[:])
            if want_quant:
                nc.vector.scalar_tensor_tensor(xq[:, c, :], xhc[c][:], QSCALE,
                                               rsb_sb[:], Alu.mult, Alu.mult)
                nc.sync.dma_start(
                    out=bounce[parity].bitcast(QDT)[c * 128 * TLOC:(c + 1) * 128 * TLOC]
                        .rearrange("(p n) -> p n", p=128),
                    in_=xq[:, c, :])
        if want_cols:
            # per-token-block rstd columns via a DRAM round-trip transpose
            nc.sync.dma_start(out=rrd[None, :], in_=rr[:].bitcast(F32))
            nc.sync.dma_start(out=rcols[:], in_=rrd.rearrange("(b p) -> p b", p=128))

    def ln_explicit(stats):
        """consume stats -> xhc (bf16, normalized): fast ordering, c-major
        consumers chase the interleaved sub/mul chain."""
        mu_ps, e2_ps = stats
        mu = small.tile([1, TLOC], F32R, tag="sm")
        var = small.tile([1, TLOC], F32, tag="sm")
        rr = small.tile([1, TLOC], F32R, tag="sm")
        nc.vector.tensor_copy(mu[:], mu_ps[:])
        nc.vector.tensor_mul(var[:], mu[:], mu[:])
        nc.vector.tensor_sub(var[:], e2_ps[:], var[:])
        _rstd_chain(var, rr)
        rsb_ps = ps_misc.tile([128, TLOC], F32, tag="pm")
        nc.tensor.matmul(rsb_ps[:], onesr[0:1, 0:128], rr[:],
                         start=True, stop=True)
        mub_ps = ps_misc.tile([128, TLOC], F32, tag="pm")
        nc.tensor.matmul(mub_ps[:], onesr[0:1, 0:128], mu[:],
                         start=True, stop=True)
        xs_ = scr.tile([128, DC, TLOC], F32, tag="s8c")
        for c in range(DC):
            nc.vector.tensor_sub(xs_[:, c, :], h[:, c, :], mub_ps[:])
            nc.vector.tensor_mul(xhc[c][:], xs_[:, c, :], rsb_ps[:])

    eng = nc.gpsimd
    pid = eng.partition_id()
    rpar = eng.alloc_register("rpar")
    eng.reg_mod(rpar, pid, 2)
    rpeer = eng.alloc_register("rpeer")
    eng.reg_alu(rpeer, 1, rpar, mybir.AluOpType.subtract)
    rxb = eng.alloc_register("rxb")
    eng.reg_mul(rxb, rpeer, X_ELEMS)
    xbase_sv = eng.snap(rxb, donate=True, min_val=0, max_val=X_ELEMS)

    def emit_wdma(i):
        """Queue the bf16 weight loads for layer i (ring-buffered pool)."""
        tiles = {}
        tiles["w1k"] = wpool.tile([128, DC, D], BF16, tag="w", name=f"w1k_{i}")
        nc.sync.dma_start(out=tiles["w1k"][:], in_=w1_d[i, :, D:2 * D]
                          .rearrange("(c p) o -> p c o", p=128))
        tiles["w1q"] = wpool.tile([128, DC, D], BF16, tag="w", name=f"w1q_{i}")
        nc.sync.dma_start(out=tiles["w1q"][:], in_=w1_d[i, :, 0:D]
                          .rearrange("(c p) o -> p c o", p=128))
        tiles["w1v"] = wpool.tile([128, DC, D], BF16, tag="w", name=f"w1v_{i}")
        nc.sync.dma_start(out=tiles["w1v"][:], in_=w1_d[i, :, 2 * D:3 * D]
                          .rearrange("(c p) o -> p c o", p=128))
        tiles["wp"] = wpool.tile([128, DC, D], BF16, tag="w", name=f"wp_{i}")
        nc.sync.dma_start(out=tiles["wp"][:], in_=wp_d[i]
                          .rearrange("(c p) o -> p c o", p=128))
        tiles["w2"] = []
        for qi in range(4):
            w2t = wpool.tile([128, DC, D], BF16, tag="w", name=f"w2_{i}_{qi}")
            nc.sync.dma_start(out=w2t[:], in_=w2_d[i, :, qi * D:(qi + 1) * D]
                              .rearrange("(c p) o -> p c o", p=128))
            tiles["w2"].append(w2t)
        tiles["w3"] = []
        for qi in range(4):
            w3t = wpool.tile([128, DC, D], BF16, tag="w", name=f"w3_{i}_{qi}")
            nc.sync.dma_start(out=w3t[:], in_=w3_d[i, qi * D:(qi + 1) * D, :]
                              .rearrange("(c p) o -> p c o", p=128))
            tiles["w3"].append(w3t)
        return tiles

    # ---- embed: h = read_w.T @ xsT + posT, with LN1 stats chasing ----
    posTt = scr.tile([128, DC, TLOC], F32, tag="s8b")
    nc.sync.dma_start(out=posTt[:], in_=posT_d.rearrange("c p t -> p c t"))
    wtiles = emit_wdma(0 % L)
    stats = new_stats("embed")
    for oc in range(DC):
        ps = ps_mm.tile([128, TLOC], F32, tag="mm")
        nc.tensor.matmul(ps[:], rw[0:NDIMS, oc * 128:(oc + 1) * 128], xsT[:],
                         start=True, stop=True)
        nc.vector.tensor_add(h[:, oc, :], ps[:], posTt[:, oc, :])
        res_square(oc)
    stats_mms(stats)

    for li in range(n_layers):
        i = li % L
        lt = 0 if i < 2 else 1
        w1k, w1v, w1q = wtiles["w1k"], wtiles["w1v"], wtiles["w1q"]
        wpt, w2q, w3q = wtiles["wp"], wtiles["w2"], wtiles["w3"]

        # ---- LN1 (deferred scale), then exchange int8 xhat with the peer ----
        with nc.named_scope(f"ln1_{li}"):
            if KDEFER:
                ln_defer(stats, parity=li % 2)
            else:
                ln_explicit(stats)
                for c in range(DC):
                    nc.vector.tensor_scalar(xq[:, c, :], xhc[c][:], QSCALE, None,
                                            Alu.mult)
                    nc.sync.dma_start(
                        out=bounce[li % 2].bitcast(QDT)[c * 128 * TLOC:(c + 1) * 128 * TLOC]
                            .rearrange("(p n) -> p n", p=128),
                        in_=xq[:, c, :])
        if fake_ag:
            nc.sync.dma_start(out=agout[li % 2, 0:X_WORDS], in_=bounce[li % 2])
            nc.sync.dma_start(out=agout[li % 2, X_WORDS:2 * X_WORDS], in_=bounce[li % 2])
        else:
            nc.gpsimd.collective_compute(
                "AllGather", mybir.AluOpType.bypass, replica_groups=PAIRS,
                ins=[bounce[li % 2]], outs=[agout[li % 2]])
        nc.gpsimd.dma_start(
            out=xrem[:],
            in_=agout[li % 2].bitcast(QDT)[bass.ds(xbase_sv, X_ELEMS)]
                .rearrange("(c p t) -> p c t", c=DC, p=128))
        # weight prefetch for the next layer rides behind the xhat DMA
        if li + 1 < n_layers:
            next_tiles = emit_wdma((li + 1) % L)

        # ---- local QKV projections (overlap the collective) ----
        # c-major emission: the first wave of matmuls needs only xhat chunk 0,
        # so the PE chases the LN chain instead of waiting for all chunks.
        kq_ps = [ps_mm.tile([128, TLOC], F32, tag="mm", name=f"kps{oc}")
                 for oc in range(DC)]
        for c in range(DC):
            for oc in range(DC):
                nc.tensor.matmul(kq_ps[oc][:], w1k[:, c, oc * 128:(oc + 1) * 128],
                                 xhc[c][:], start=(c == 0),
                                 stop=(c == DC - 1))
        for oc in range(DC):
            if KDEFER:
                nc.vector.tensor_mul(kst[:, oc, :], kq_ps[oc][:], rsb_sb[:])
            else:
                nc.scalar.copy(out=kst[:, oc, :], in_=kq_ps[oc][:])
        kq_ps = [ps_mm.tile([128, TLOC], F32, tag="mm", name=f"qps{oc}")
                 for oc in range(DC)]
        for c in range(DC):
            for oc in range(DC):
                nc.tensor.matmul(kq_ps[oc][:], w1q[:, c, oc * 128:(oc + 1) * 128],
                                 xhc[c][:], start=(c == 0),
                                 stop=(c == DC - 1))
        for oc in range(DC):
            if KDEFER:
                nc.vector.tensor_mul(qT[:, oc, :], kq_ps[oc][:], rsb_sb[:])
            else:
                nc.scalar.copy(out=qT[:, oc, :], in_=kq_ps[oc][:])
        kq_ps = [ps_mm.tile([128, TLOC], F32, tag="mm", name=f"vps{oc}")
                 for oc in range(DC)]
        for c in range(DC):
            for tcb in range(4):
                nc.tensor.matmul(kq_ps[tcb][:], xhc[c][:, tcb * 128:(tcb + 1) * 128],
                                 w1v[:, c, :], start=(c == 0),
                                 stop=(c == DC - 1))
        for tcb in range(4):
            if KDEFER:
                nc.vector.tensor_scalar(
                    vloc[:, tcb, :, 0:HD], kq_ps[tcb][:].rearrange("p (h d) -> p h d", h=H),
                    rcols[:, tcb:tcb + 1], None, Alu.mult)
            else:
                nc.vector.tensor_copy(
                    vloc[:, tcb, :, 0:HD], kq_ps[tcb][:].rearrange("p (h d) -> p h d", h=H))

        # ---- attention pass 1: local key blocks, all heads ----
        for hh in range(H):
            hc, hr = hh // 2, (hh % 2) * HD
            cps = ps_cps.tile([HD + 1, TLOC], F32, tag="cps")
            for vi in range(4):
                s, w = vi, NPAD_V[vi]
                sps = ps_mm.tile([128, TLOC], F32, tag="mm")
                nc.tensor.matmul(sps[:, 0:w],
                                 kst[hr:hr + HD, hc, s * 128:(s + 1) * 128],
                                 qT[hr:hr + HD, hc, TLOC - w:TLOC],
                                 start=True, stop=True)
                pt = ppool.tile([128, TLOC], BF16, tag="P")
                nc.scalar.activation(out=pt[:, 0:w], in_=sps[:, 0:w],
                                     func=AF.Exp, scale=0.125)
                mw = w if (lt == 0 and s == 0) else 128
                nc.gpsimd.tensor_mul(pt[:, 0:mw], pt[:, 0:mw],
                                     mt[:, lt, OFF[vi]:OFF[vi] + mw])
                nc.tensor.matmul(cps[:, TLOC - w:TLOC], vloc[:, s, hh, :],
                                 pt[:, 0:w], start=(vi == 0), stop=(vi == 3))
            nc.vector.tensor_copy(ctxl[:, hh, :], cps[:])

        # ---- dequant + peer K/V projections from the gathered xhat ----
        for c in range(DC):
            nc.vector.tensor_scalar(xrb[c][:], xrem[:, c, :], 1.0 / QSCALE, None,
                                    Alu.mult)
        if dbg_xrem is not None and li == 0:
            nc.sync.dma_start(out=dbg_xrem[:], in_=xrem[:].rearrange("p c t -> p (c t)"))
            for c in range(DC):
                nc.sync.dma_start(out=dbg_xrb[:, c * TLOC:(c + 1) * TLOC], in_=xrb[c][:])
        kq_ps = [ps_mm.tile([128, TLOC], F32, tag="mm", name=f"keps{oc}")
                 for oc in range(DC)]
        for c in range(DC):
            for oc in range(DC):
                nc.tensor.matmul(kq_ps[oc][:], w1k[:, c, oc * 128:(oc + 1) * 128],
                                 xrb[c][:], start=(c == 0),
                                 stop=(c == DC - 1))
        for oc in range(DC):
            nc.scalar.copy(out=kpe[:, oc, :], in_=kq_ps[oc][:])
        kq_ps = [ps_mm.tile([128, TLOC], F32, tag="mm", name=f"veps{oc}")
                 for oc in range(DC)]
        for c in range(DC):
            for tcb in range(4):
                nc.tensor.matmul(kq_ps[tcb][:], xrb[c][:, tcb * 128:(tcb + 1) * 128],
                                 w1v[:, c, :], start=(c == 0),
                                 stop=(c == DC - 1))
        for tcb in range(4):
            nc.vector.tensor_copy(
                vpe[:, tcb, :, 0:HD], kq_ps[tcb][:].rearrange("p (h d) -> p h d", h=H))

        if d_kpe is not None and li == 0:
            nc.sync.dma_start(out=d_kpe[:], in_=kpe[:].rearrange("p c t -> p (c t)"))
            nc.sync.dma_start(out=d_vpe[:], in_=vpe[:].rearrange("p a h e -> p (a h e)"))
            nc.sync.dma_start(out=d_kst[:], in_=kst[:].rearrange("p c t -> p (c t)"))
            nc.sync.dma_start(out=d_qT[:], in_=qT[:].rearrange("p c t -> p (c t)"))
            nc.sync.dma_start(out=d_vloc[:], in_=vloc[:].rearrange("p a h e -> p (a h e)"))
        # ---- attention pass 2: remote key blocks + normalize ----
        for hh in range(H):
            hc, hr = hh // 2, (hh % 2) * HD
            cps = ps_cps.tile([HD + 1, TLOC], F32, tag="cps")
            for vi in range(4, NB):
                s, w = vi % 4, NPAD_V[vi]
                sps = ps_mm.tile([128, TLOC], F32, tag="mm")
                nc.tensor.matmul(sps[:, 0:w],
                                 kpe[hr:hr + HD, hc, s * 128:(s + 1) * 128],
                                 qT[hr:hr + HD, hc, TLOC - w:TLOC],
                                 start=True, stop=True)
                pt = ppool.tile([128, TLOC], BF16, tag="P")
                nc.scalar.activation(out=pt[:, 0:w], in_=sps[:, 0:w],
                                     func=AF.Exp, scale=0.125)
                mw = w if (lt == 0 and s == 0) else 128
                nc.gpsimd.tensor_mul(pt[:, 0:mw], pt[:, 0:mw],
                                     mt[:, lt, OFF[vi]:OFF[vi] + mw])
                nc.tensor.matmul(cps[:, TLOC - w:TLOC], vpe[:, s, hh, :],
                                 pt[:, 0:w], start=(vi == 4), stop=(vi == 7))
            nc.vector.tensor_add(ctxl[:, hh, :], ctxl[:, hh, :], cps[:])
            rec = small.tile([1, TLOC], F32R, tag="sm")
            with nc.allow_low_precision(reason="f32r softmax denom recip"):
                nc.vector.reciprocal(rec[:], ctxl[HD:HD + 1, hh, :])
            rb = ps_misc.tile([HD, TLOC], F32, tag="pm", name=f"rb{hh}")
            nc.tensor.matmul(rb[:], onesr[0:1, 0:HD], rec[:], start=True, stop=True)
            nc.vector.tensor_mul(ctxf[hr:hr + HD, hc, :], ctxl[0:HD, hh, :], rb[:])
        # preload the gelu table while the out-projection runs
        nc.scalar.activation(out=scr1[:], in_=epst[:].bitcast(F32),
                             func=AF.Gelu_apprx_tanh, bias=0.0)

        if d_ctxf is not None and li == 0:
            nc.sync.dma_start(out=d_ctxf[:], in_=ctxf[:].rearrange("p c t -> p (c t)"))
            nc.sync.dma_start(out=d_xq[:], in_=xq[:].rearrange("p c t -> p (c t)"))
        # ---- attention out-projection + residual, LN2 stats chasing ----
        stats = new_stats(f"l{li}a")
        pr_ps = [ps_mm.tile([128, TLOC], F32, tag="mm", name=f"pps{oc}")
                 for oc in range(DC)]
        for c in range(DC):
            for oc in range(DC):
                nc.tensor.matmul(pr_ps[oc][:], wpt[:, c, oc * 128:(oc + 1) * 128],
                                 ctxf[:, c, :], start=(c == 0),
                                 stop=(c == DC - 1))
        for oc in range(DC):
            nc.vector.tensor_add(h[:, oc, :], h[:, oc, :], pr_ps[oc][:])
            res_square(oc)
        stats_mms(stats)

        # ---- LN2 + MLP ----
        with nc.named_scope(f"ln2_{li}"):
            ln_explicit(stats)
        for g in range(4):
            fc_ps = [ps_mm.tile([128, TLOC], F32, tag="mm", name=f"fps{g}_{j}")
                     for j in range(4)]
            for c in range(DC):
                for j in range(4):
                    oc = g * 4 + j
                    nc.tensor.matmul(fc_ps[j][:],
                                     w2q[oc // 4][:, c, (oc % 4) * 128:(oc % 4 + 1) * 128],
                                     xhc[c][:], start=(c == 0),
                                     stop=(c == DC - 1))
            for j in range(4):
                oc = g * 4 + j
                nc.scalar.activation(out=gel[:, oc, :], in_=fc_ps[j][:],
                                     func=AF.Gelu_apprx_tanh, bias=0.0)
        # preload the exp table while the down-projection runs
        nc.scalar.activation(out=scr1[:], in_=epst[:].bitcast(F32),
                             func=AF.Exp, scale=1.0)
        stats = new_stats(f"l{li}m")
        for oc in range(DC):
            pp = ps_mm.tile([128, TLOC], F32, tag="mm")
            for kc in range(16):
                nc.tensor.matmul(pp[:], w3q[kc // 4][:, kc % 4, oc * 128:(oc + 1) * 128],
                                 gel[:, kc, :], start=(kc == 0),
                                 stop=(kc == 15))
            nc.vector.tensor_add(h[:, oc, :], h[:, oc, :], pp[:])
            res_square(oc)
        stats_mms(stats)
        if li + 1 < n_layers:
            wtiles = next_tiles

    # ---- final LN + vocab projection ----
    ln_explicit(stats)
    ps = ps_mm.tile([VOCAB, TLOC], F32, tag="mm")
    for c in range(DC):
        nc.tensor.matmul(ps[:], wot[:, c, :], xhc[c][:],
                         start=(c == 0), stop=(c == DC - 1))
    osb = opool.tile([VOCAB, TLOC], F32, tag="osb")
    nc.scalar.copy(out=osb[:], in_=ps[:])
    nc.sync.dma_start(out=out_d[:], in_=osb[:])
    ctx.close()


def _valid_full():
    """valid[lt, k, q] over global token ids."""
    q = np.arange(S)[None, :]
    k = np.arange(S)[:, None]
    causal = k <= q
    # layer type 0 (mask_first)
    blk = (k // 4 == q // 4) & (q < 20) & (k < 20)
    row20 = (q == 20) & (k <= 20)
    path0 = (q >= SCHEMA) & (k >= SCHEMA)
    m0 = (blk | row20 | path0) & causal
    return np.stack([m0, causal])


def _prep(inputs):
    f32 = lambda a: np.ascontiguousarray(np.asarray(a), dtype=np.float32)
    bf = ml_dtypes.bfloat16
    xs = f32(inputs["xs"])
    read_w, read_b = f32(inputs["read_w"]), f32(inputs["read_b"])
    pos = np.concatenate([f32(inputs["pos_schema"]),
                          f32(inputs["pos_path"])[: S - SCHEMA]], axis=0)
    ln1_g = f32(inputs["ln1_g"])
    ln2_g = f32(inputs["ln2_g"])
    lnf_g = f32(inputs["lnf_g"])
    attn_w = f32(inputs["attn_w"])
    attnp_w = f32(inputs["attnp_w"])
    fc_w = f32(inputs["fc_w"])
    proj_w = f32(inputs["proj_w"])
    out_w = f32(inputs["out_w"])

    # biases are all zero for this model; LN gains fold into the weights
    ln1_b, attn_b = f32(inputs["ln1_b"]), f32(inputs["attn_b"])
    ln2_b, fc_b = f32(inputs["ln2_b"]), f32(inputs["fc_b"])
    lnf_b = f32(inputs["lnf_b"])
    b1 = np.einsum("ld,ldo->lo", ln1_b, attn_w) + attn_b
    b2 = np.einsum("ld,ldo->lo", ln2_b, fc_w) + fc_b
    bo = lnf_b @ out_w + f32(inputs["out_b"])
    assert not any(np.any(v) for v in
                   (b1, f32(inputs["attnp_b"]), b2, f32(inputs["proj_b"]), bo)), \
        "bias-free fast path: nonzero biases not supported"

    w1 = (attn_w * ln1_g[:, :, None]).astype(bf)
    w2 = (fc_w * ln2_g[:, :, None]).astype(bf)
    wo = (out_w * lnf_g[:, None]).astype(bf)
    wp = attnp_w.astype(bf)
    w3 = proj_w.astype(bf)

    valid = _valid_full()
    shared = dict(rw=read_w.astype(np.float32), w1=w1, wp=wp,
                  w2=w2, w3=w3, wo=wo)

    in_maps = []
    for c in range(8):
        b = c // 2
        blocks = H0_BLOCKS if c % 2 == 0 else H1_BLOCKS
        toks = np.concatenate([np.arange(bb * TB, (bb + 1) * TB) for bb in blocks])
        xsT = np.ascontiguousarray(xs[b][toks].T)                    # (64, 512)
        posT = (pos[toks] + read_b[None, :]).T                        # (512, 512)
        posT = np.ascontiguousarray(posT.reshape(DC, 128, TLOC))
        peer_blocks = H1_BLOCKS if c % 2 == 0 else H0_BLOCKS
        vslot_blocks = list(blocks) + list(peer_blocks)
        masks = np.zeros((2, 128, SUM_NPAD), dtype=bf)
        for lt in range(2):
            for vi, j in enumerate(vslot_blocks):
                w = NPAD_V[vi]
                cols = toks[TLOC - w:]
                masks[lt, :, OFF[vi]:OFF[vi] + w] = \
                    valid[lt, j * TB:(j + 1) * TB][:, cols].astype(bf)
        m = dict(shared)
        m.update(xsT=xsT, posT=posT, masks=masks)
        in_maps.append(m)
    return in_maps


def kernel(**inputs):
    global LAST_RESULTS
    in_maps = _prep(inputs)
    key = L
    if key not in _PROGRAM_CACHE:
        _PROGRAM_CACHE[key] = _build_program(L)
    nc = _PROGRAM_CACHE[key]
    bench = int(os.environ.get("KBENCH_REPS", "0"))
    results = _run_spmd(nc, in_maps, bench_reps=bench)
    LAST_RESULTS = results

    out = np.zeros((4, S, VOCAB), dtype=np.float32)
    for c in range(8):
        b = c // 2
        blocks = H0_BLOCKS if c % 2 == 0 else H1_BLOCKS
        o = results[c]["outT"]                                        # (19, 512)
        for bi, bb in enumerate(blocks):
            out[b, bb * TB:(bb + 1) * TB, :] = o[:, bi * TB:(bi + 1) * TB].T
    return out
